# revision 21
# baseline (speedup 1.0000x reference)
"""Trainium2 Bass kernel for the CerealBar VIN problem.

Self-contained: hardcodes shapes B=512, E=25, 6 orientations, gamma=0.9,
8-core batch sharding (64 samples/core).

Math (bias-trick formulation, derived from the reference):
  The grouped 3x3 conv is a set of one-hot spatial shifts. Encode
  obstacles / out-of-grid cells as a -1000 bias folded into the goal map:
    gmB = goals  where free,  -1000 where masked.
  Iterate on W directly (W_{-1} = gmB):
    Z[o]   = max(sh_{+d(o)}(W[o]), sh_{-d(o)}(W[o]), W[o+1], W[o-1])
    W'[o]  = gamma * Z[o] + gmB[o]          (one scalar_tensor_tensor)
  On free cells all values stay >= 0, so the -1000 terms never win a max
  and W equals the reference's masked W exactly; masked cells drift very
  negative and the host clamps them to 0 at gather time. This needs 4
  vector passes/iter vs 5 for the direct masked form (no m-multiply, no
  separate gamma scale) and zero on-chip setup compute.

Device layout: partition p = h*64 + i -> sample i of the core, row-half h.
Each (orientation, half) plane = 21 rows x 26 cols flat (546): row 0 top
halo, rows 1..19 data, row 20 bottom halo, col 25 pad. half0 data = grid
rows 0..18, half1 = grid rows 19..37. Shifts are free-dim AP offsets
(d = 26*dy+dx); the row-split halo rows are refreshed once per iteration
by two cross-partition SBUF-SBUF DMAs kicked right after the boundary
rows (1, 19) of W are built, hidden under ~4us of halo-independent work.
"""
import json
import sys

sys.path.insert(0, "/opt/trn_rl_repo")

import numpy as np

import concourse.bass as bass
import concourse.mybir as mybir
from concourse.ap import AP
from concourse.bass_utils import run_bass_kernel_spmd
from concourse.tile import TileContext

E = 25
ADD = 12
GAMMA = 0.9
BIG = -100.0     # bias for masked cells
PR = 40          # padded full-grid rows (grid rows -1..38 at idx r+1)
PC = 26
SLOT = 546       # 21 * 26 per half-plane
DOF = 26         # data offset within a slot (row 1)
DN = 494         # data elems (rows 1..19)
N_CORES = 8
BPC = 64         # samples per core

# shifts (dy, dx): out[y, x] = in[y+dy, x+dx]
D0 = [(0, 1), (1, 0), (1, -1), (0, -1), (-1, 0), (-1, 1)]
PAIRS = [(0, 3), (1, 2), (4, 5)]  # (0,3) first: dy=0, no halo-row dep

import os as _os

if _os.environ.get("KDT", "fp16") == "fp16":
    DTYPE = mybir.dt.float16
    NP_DT = np.float16
else:
    DTYPE = mybir.dt.float32
    NP_DT = np.float32

TRACE = False
LAST_RESULT = None

_u = np.arange(E)[:, None]
_v = np.arange(E)[None, :]
_ROW = (_u - _v // 2 + ADD) + 1
_COL = np.broadcast_to(_v, (E, E))


# ---------------------------------------------------------------- BIR fixups
def _split_multi_waits(bir):
    """The installed walrus rejects >1 sync wait per instruction; hoist
    extras onto single-wait NoOps inserted before it on the same engine."""
    for fn in bir.get("functions", []):
        for blk in fn.get("blocks", []):
            out = []
            for ins in blk.get("instructions", []):
                si = ins.get("sync_info")
                waits = (si or {}).get("on_wait") or []
                if len(waits) > 1:
                    for k, w in enumerate(waits[:-1]):
                        out.append({
                            "debug": ins.get("debug", 0),
                            "engine": ins["engine"],
                            "ins": [], "outs": [],
                            "name": f"{ins['name']}_w{k}",
                            "opcode": "NoOp",
                            "sync_info": {"on_wait": [w], "on_update": []},
                            "text_hint": "split_wait",
                        })
                    si["on_wait"] = [waits[-1]]
                out.append(ins)
            blk["instructions"] = out
    return bir


def _install_compat(nc):
    orig = nc.to_json_bytes

    def patched():
        return json.dumps(_split_multi_waits(json.loads(orig()))).encode()

    nc.to_json_bytes = patched


# ---------------------------------------------------------------- kernel build
def _rap(t, off, pairs):
    """Raw AP over pool tile t (full 128 partitions) with free dims pairs."""
    return AP(t.tensor, int(t.offset) + off, [list(t.ap[0])] + [list(p) for p in pairs])


def _delta(d):
    return 26 * d[0] + d[1]


def build_nc(n_iter):
    nc = bass.Bass()
    _install_compat(nc)
    mx = mybir.AluOpType.max
    add = mybir.AluOpType.add

    gi_d = nc.declare_dram_parameter("ginit", [128, 6, SLOT], DTYPE, isOutput=False)
    gs_d = nc.declare_dram_parameter("gs", [128, n_iter, 6, SLOT], DTYPE,
                                     isOutput=False)
    w_d = nc.declare_dram_parameter("w", [128, 6, SLOT], DTYPE, isOutput=True)

    with TileContext(nc) as tc:
        with tc.tile_pool(name="p", bufs=1) as pool:
            ginit = pool.tile([128, 6, SLOT], DTYPE)
            gs = pool.tile([128, n_iter, 6, SLOT], DTYPE)
            wb = pool.tile([128, 6, SLOT], DTYPE)
            t0 = pool.tile([128, 6, SLOT], DTYPE)   # X, then Z in place
            t1 = pool.tile([128, 6, SLOT], DTYPE)   # M2

            # Y_{-1} gates iteration 0 -- its DMA goes first, then the first
            # two G tiles. All streaming goes on the sync queue in small
            # per-iteration chunks: DMA engines drain descriptors FIFO, so a
            # big chunk would block the latency-critical halo DMAs behind it.
            nc.sync.dma_start(out=ginit[:, 0:3], in_=gi_d[:, 0:3])
            nc.sync.dma_start(out=ginit[:, 3:6], in_=gi_d[:, 3:6])
            for k in range(min(2, n_iter)):
                nc.sync.dma_start(out=gs[:, k], in_=gs_d[:, k])
            # fake halo rows (grid -1 / grid 38) stay at BIG forever
            nc.gpsimd.memset(wb[0:64, :, 0:26], BIG)
            nc.gpsimd.memset(wb[64:128, :, 20 * 26:21 * 26], BIG)

            def x_pair(src, oa, ob):
                # t0[{oa,ob}] = max(sh_{+d0}(src), sh_{-d0}(src))
                da, db = _delta(D0[oa]), _delta(D0[ob])
                step = (ob - oa) * SLOT
                in0 = _rap(src, oa * SLOT + DOF + da, [[step + (db - da), 2], [1, DN]])
                in1 = _rap(src, oa * SLOT + DOF - da, [[step - (db - da), 2], [1, DN]])
                out = _rap(t0, oa * SLOT + DOF, [[step, 2], [1, DN]])
                nc.vector.tensor_tensor(out=out, in0=in0, in1=in1, op=mx)

            dv = (slice(None), slice(None), slice(DOF, DOF + DN))

            def halo():
                # half1 top halo <- half0 grid row 18 (buffer row 19)
                nc.sync.dma_start(out=wb[64:128, :, 0:26],
                                  in_=wb[0:64, :, 19 * 26:20 * 26])
                # half0 bottom halo <- half1 grid row 19 (buffer row 1)
                nc.sync.dma_start(out=wb[0:64, :, 20 * 26:21 * 26],
                                  in_=wb[64:128, :, 26:52])

            for it in range(n_iter):
                src = ginit if it == 0 else wb
                # halo-independent ops first so the previous iteration's
                # halo DMA has the whole window to land. On iteration 0,
                # X(1,2) goes first: it only needs the first ginit chunk.
                if it == 0:
                    x_pair(src, 1, 2)
                    x_pair(src, 0, 3)
                else:
                    x_pair(src, 0, 3)
                # M2: middle slots batched, then wrap slots {0,5}
                nc.vector.tensor_tensor(
                    out=t1[:, 1:5, DOF:DOF + DN], in0=src[:, 2:6, DOF:DOF + DN],
                    in1=src[:, 0:4, DOF:DOF + DN], op=mx)
                nc.vector.tensor_tensor(
                    out=_rap(t1, DOF, [[5 * SLOT, 2], [1, DN]]),
                    in0=_rap(src, SLOT + DOF, [[-SLOT, 2], [1, DN]]),
                    in1=_rap(src, 5 * SLOT + DOF, [[-SLOT, 2], [1, DN]]), op=mx)
                if it == 0:
                    x_pair(src, 4, 5)
                else:
                    x_pair(src, 1, 2)
                    x_pair(src, 4, 5)
                # Z = max(X, M2) in place in t0
                nc.vector.tensor_tensor(out=t0[dv], in0=t0[dv], in1=t1[dv], op=mx)
                # Y' = Z + G_it (pure adds at fp16 2x rate); boundary rows
                # (1, 19) go on the idle gpsimd engine (2 free dims max per
                # op there) so the vector starts the interior immediately
                goff = it * 6 * SLOT
                for roff in (26, 19 * 26):
                    wa = _rap(wb, roff, [[SLOT, 6], [1, 26]])
                    za = _rap(t0, roff, [[SLOT, 6], [1, 26]])
                    ga = _rap(gs, goff + roff, [[SLOT, 6], [1, 26]])
                    nc.gpsimd.tensor_tensor(out=wa, in0=za, in1=ga, op=add)
                if it < n_iter - 1:
                    halo()
                    # stream the G tile two iterations ahead, after the halo
                    # DMAs so it can never delay them in the engine FIFOs
                    if it + 2 < n_iter:
                        nc.sync.dma_start(out=gs[:, it + 2], in_=gs_d[:, it + 2])
                    wi = _rap(wb, 52, [[SLOT, 6], [1, 442]])
                    zi = _rap(t0, 52, [[SLOT, 6], [1, 442]])
                    gi = _rap(gs, goff + 52, [[SLOT, 6], [1, 442]])
                    nc.vector.tensor_tensor(out=wi, in0=zi, in1=gi, op=add)
                else:
                    # final iteration: build + ship out in slot halves so
                    # the two output DMAs overlap the remaining adds
                    for s0 in (0, 3):
                        wi = _rap(wb, s0 * SLOT + 52, [[SLOT, 3], [1, 442]])
                        zi = _rap(t0, s0 * SLOT + 52, [[SLOT, 3], [1, 442]])
                        gi = _rap(gs, goff + s0 * SLOT + 52, [[SLOT, 3], [1, 442]])
                        nc.vector.tensor_tensor(out=wi, in0=zi, in1=gi, op=add)
                        eng = nc.sync if s0 == 0 else nc.scalar
                        eng.dma_start(out=w_d[:, s0:s0 + 3], in_=wb[:, s0:s0 + 3])
    return nc


_NC_CACHE = {}


def _get_nc(n_iter):
    if n_iter not in _NC_CACHE:
        _NC_CACHE[n_iter] = build_nc(n_iter)
    return _NC_CACHE[n_iter]


# ---------------------------------------------------------------- host side
def _to_padded_axial(x):
    out = np.zeros(x.shape[:-2] + (PR, PC), np.float32)
    out[..., _ROW, _COL] = x
    return out


def kernel(offset_input_goals, offset_current_state, offset_obstacles,
           num_iterations):
    global LAST_RESULT
    goals = np.asarray(offset_input_goals, np.float32)
    state = np.asarray(offset_current_state)
    obst = np.asarray(offset_obstacles, np.float32)
    n_iter = int(num_iterations)
    B = goals.shape[0]
    assert B == N_CORES * BPC and n_iter >= 1

    goals_ax = _to_padded_axial(goals)                     # [B,6,40,26]
    mask = _to_padded_axial(np.ones((E, E), np.float32))
    m_full = (1.0 - _to_padded_axial(obst)) * mask         # [B,40,26]
    gmb_full = np.where(m_full[:, None] > 0.5, goals_ax, BIG)  # [B,6,40,26]

    def split(x):  # [B, ..., 40, 26] -> [B, ..., 546] halves
        h0 = x[..., 0:21, :].reshape(x.shape[:-2] + (SLOT,))
        h1 = x[..., 19:40, :].reshape(x.shape[:-2] + (SLOT,))
        return h0, h1

    g0, g1 = split(gmb_full)
    gmb_h = np.stack([g0, g1], 1)                          # [B,2,6,546]
    # rescaled-domain goal tensors: Y_k = W_k / gamma^(k+1) turns the
    # update into  Y' = max-tree(Y) + G_k,  G_k = gmb * gamma^-(k+1)
    scales = GAMMA ** -(np.arange(1, n_iter + 1, dtype=np.float32))
    gs_h = (gmb_h[:, :, None] * scales[None, None, :, None, None]).astype(NP_DT)

    in_maps = []
    for c in range(N_CORES):
        s = slice(c * BPC, (c + 1) * BPC)
        gi = np.concatenate([g0[s], g1[s]], 0).astype(NP_DT)
        gsc = np.concatenate([gs_h[s, 0], gs_h[s, 1]], 0)  # [128,n_iter,6,546]
        in_maps.append({"ginit": gi, "gs": gsc})

    nc = _get_nc(n_iter)
    res = run_bass_kernel_spmd(nc, in_maps, core_ids=list(range(N_CORES)),
                               trace=TRACE)
    LAST_RESULT = res

    w_all = np.stack([np.asarray(res.results[c]["w"], np.float32)
                      for c in range(N_CORES)], 0)         # [8,128,6,546]

    alpha = state[:, 0].astype(np.int64)
    uu = (state[:, 1] - state[:, 2] // 2 + ADD).astype(np.int64)  # grid row
    vv = state[:, 2].astype(np.int64)
    rot = (alpha + 1) % 6
    bs = np.arange(B)
    core = bs // BPC
    lane = bs % BPC

    w_scale = np.float32(GAMMA ** n_iter)   # W_final = Y_final * gamma^n

    def read_w(slot, g, c):
        # clamped gather of W at grid row g, col c (0 outside grid / masked)
        valid = (g >= 0) & (g <= 37) & (c >= 0) & (c <= 24)
        h = (g > 18).astype(np.int64)
        local = np.where(h == 1, g - 18, g + 1)
        p = h * 64 + lane
        idx = np.clip(local * 26 + c, 0, SLOT - 1)
        val = w_all[core, p, slot, idx] * w_scale
        return np.where(valid, np.maximum(val, 0.0), 0.0)

    dy0 = np.array([d[0] for d in D0])[rot]
    dx0 = np.array([d[1] for d in D0])[rot]
    m_pt = m_full[bs, uu + 1, vv]

    out = np.zeros((B, 4), np.float32)
    out[:, 0] = m_pt * read_w(rot, uu + dy0, vv + dx0)
    out[:, 1] = m_pt * read_w(rot, uu - dy0, vv - dx0)
    out[:, 2] = read_w((rot + 1) % 6, uu, vv)
    out[:, 3] = read_w((rot + 5) % 6, uu, vv)
    return out


# revision 23
# speedup vs baseline: 1.1823x; 1.1823x over previous
"""Trainium2 Bass kernel for the CerealBar VIN problem.

Self-contained: hardcodes shapes B=512, E=25, 6 orientations, gamma=0.9,
8-core batch sharding (64 samples/core).

Math (bias trick + rescaled domain, derived from the reference):
  The grouped 3x3 conv is a set of one-hot spatial shifts. Encode
  obstacles / out-of-grid cells as a -100 bias folded into the goal map
  (gmB), and iterate in the rescaled domain Y_k = W_k / gamma^(k+1):
    Z[o]   = max(sh_{+d(o)}(Y[o]), sh_{-d(o)}(Y[o]), Y[o+1], Y[o-1])
    Y'[o]  = Z[o] + G_k[o],   G_k = gmB * gamma^-(k+1)  (host-prescaled)
  On free cells all values stay >= 0, so the -100 bias terms never win a
  max and Y tracks the reference's masked W exactly (host multiplies by
  gamma^n and clamps at gather time); masked cells just drift negative.
  Every pass is a plain max or add, all of which run in the DVE's fp16
  2x mode (0.52 ns/elem) -- scalar_tensor_tensor would run at 1x, which
  is why gamma is folded into the streamed G_k tensors instead. 4 vector
  passes/iter (3 max-tree + 1 add), 8 instructions, zero setup compute.
  The 21 G tensors (137 KB/partition) stream from HBM on the sync queue
  in one-iteration chunks issued after each halo pair -- DMA engines
  drain descriptors FIFO, so big chunks would block the halo DMAs.

Device layout: partition p = h*64 + i -> sample i of the core, row-half h.
Each (orientation, half) plane = 21 rows x 26 cols flat (546): row 0 top
halo, rows 1..19 data, row 20 bottom halo, col 25 pad. half0 data = grid
rows 0..18, half1 = grid rows 19..37. Shifts are free-dim AP offsets
(d = 26*dy+dx); the row-split halo rows are refreshed once per iteration
by two cross-partition SBUF-SBUF DMAs kicked right after the boundary
rows (1, 19) of W are built, hidden under ~4us of halo-independent work.
"""
import json
import sys

sys.path.insert(0, "/opt/trn_rl_repo")

import numpy as np

import concourse.bass as bass
import concourse.mybir as mybir
from concourse.ap import AP
from concourse.bass_utils import run_bass_kernel_spmd
from concourse.tile import TileContext

E = 25
ADD = 12
GAMMA = 0.9
BIG = -100.0     # bias for masked cells
PR = 40          # padded full-grid rows (grid rows -1..38 at idx r+1)
PC = 26
SLOT = 546       # 21 * 26 per half-plane
DOF = 26         # data offset within a slot (row 1)
DN = 494         # data elems (rows 1..19)
N_CORES = 8
BPC = 64         # samples per core

# shifts (dy, dx): out[y, x] = in[y+dy, x+dx]
D0 = [(0, 1), (1, 0), (1, -1), (0, -1), (-1, 0), (-1, 1)]
PAIRS = [(0, 3), (1, 2), (4, 5)]  # (0,3) first: dy=0, no halo-row dep

import os as _os

if _os.environ.get("KDT", "fp16") == "fp16":
    DTYPE = mybir.dt.float16
    NP_DT = np.float16
else:
    DTYPE = mybir.dt.float32
    NP_DT = np.float32

TRACE = False
LAST_RESULT = None

_u = np.arange(E)[:, None]
_v = np.arange(E)[None, :]
_ROW = (_u - _v // 2 + ADD) + 1
_COL = np.broadcast_to(_v, (E, E))


# ---------------------------------------------------------------- BIR fixups
def _split_multi_waits(bir):
    """The installed walrus rejects >1 sync wait per instruction; hoist
    extras onto single-wait NoOps inserted before it on the same engine."""
    for fn in bir.get("functions", []):
        for blk in fn.get("blocks", []):
            out = []
            for ins in blk.get("instructions", []):
                si = ins.get("sync_info")
                waits = (si or {}).get("on_wait") or []
                if len(waits) > 1:
                    for k, w in enumerate(waits[:-1]):
                        out.append({
                            "debug": ins.get("debug", 0),
                            "engine": ins["engine"],
                            "ins": [], "outs": [],
                            "name": f"{ins['name']}_w{k}",
                            "opcode": "NoOp",
                            "sync_info": {"on_wait": [w], "on_update": []},
                            "text_hint": "split_wait",
                        })
                    si["on_wait"] = [waits[-1]]
                out.append(ins)
            blk["instructions"] = out
    return bir


def _install_compat(nc):
    orig = nc.to_json_bytes

    def patched():
        return json.dumps(_split_multi_waits(json.loads(orig()))).encode()

    nc.to_json_bytes = patched


# ---------------------------------------------------------------- kernel build
def _rap(t, off, pairs):
    """Raw AP over pool tile t (full 128 partitions) with free dims pairs."""
    return AP(t.tensor, int(t.offset) + off, [list(t.ap[0])] + [list(p) for p in pairs])


def _delta(d):
    return 26 * d[0] + d[1]


def build_nc(n_iter):
    nc = bass.Bass()
    _install_compat(nc)
    mx = mybir.AluOpType.max
    add = mybir.AluOpType.add

    gi_d = nc.declare_dram_parameter("ginit", [128, 6, SLOT], DTYPE, isOutput=False)
    gs_d = nc.declare_dram_parameter("gs", [128, n_iter, 6, SLOT], DTYPE,
                                     isOutput=False)
    w_d = nc.declare_dram_parameter("w", [128, 6, SLOT], DTYPE, isOutput=True)

    with TileContext(nc) as tc:
        with tc.tile_pool(name="p", bufs=1) as pool:
            ginit = pool.tile([128, 6, SLOT], DTYPE)
            gs = pool.tile([128, n_iter, 6, SLOT], DTYPE)
            wb = pool.tile([128, 6, SLOT], DTYPE)
            t0 = pool.tile([128, 6, SLOT], DTYPE)   # X, then Z in place
            t1 = pool.tile([128, 6, SLOT], DTYPE)   # M2

            # Y_{-1} gates iteration 0 -- its DMA goes first, then the first
            # two G tiles. All streaming goes on the sync queue in small
            # per-iteration chunks: DMA engines drain descriptors FIFO, so a
            # big chunk would block the latency-critical halo DMAs behind it.
            nc.sync.dma_start(out=ginit[:, 0:3], in_=gi_d[:, 0:3])
            nc.sync.dma_start(out=ginit[:, 3:6], in_=gi_d[:, 3:6])
            for k in range(min(2, n_iter)):
                nc.sync.dma_start(out=gs[:, k], in_=gs_d[:, k])
            # fake halo rows (grid -1 / grid 38) stay at BIG forever
            nc.gpsimd.memset(wb[0:64, :, 0:26], BIG)
            nc.gpsimd.memset(wb[64:128, :, 20 * 26:21 * 26], BIG)

            def x_pair(src, oa, ob):
                # t0[{oa,ob}] = max(sh_{+d0}(src), sh_{-d0}(src))
                da, db = _delta(D0[oa]), _delta(D0[ob])
                step = (ob - oa) * SLOT
                in0 = _rap(src, oa * SLOT + DOF + da, [[step + (db - da), 2], [1, DN]])
                in1 = _rap(src, oa * SLOT + DOF - da, [[step - (db - da), 2], [1, DN]])
                out = _rap(t0, oa * SLOT + DOF, [[step, 2], [1, DN]])
                nc.vector.tensor_tensor(out=out, in0=in0, in1=in1, op=mx)

            dv = (slice(None), slice(None), slice(DOF, DOF + DN))

            def halo():
                # half1 top halo <- half0 grid row 18 (buffer row 19)
                nc.sync.dma_start(out=wb[64:128, :, 0:26],
                                  in_=wb[0:64, :, 19 * 26:20 * 26])
                # half0 bottom halo <- half1 grid row 19 (buffer row 1)
                nc.sync.dma_start(out=wb[0:64, :, 20 * 26:21 * 26],
                                  in_=wb[64:128, :, 26:52])

            for it in range(n_iter):
                src = ginit if it == 0 else wb
                # halo-independent ops first so the previous iteration's
                # halo DMA has the whole window to land. On iteration 0,
                # X(1,2) goes first: it only needs the first ginit chunk.
                if it == 0:
                    x_pair(src, 1, 2)
                    x_pair(src, 0, 3)
                else:
                    x_pair(src, 0, 3)
                # M2: middle slots batched, then wrap slots {0,5}
                nc.vector.tensor_tensor(
                    out=t1[:, 1:5, DOF:DOF + DN], in0=src[:, 2:6, DOF:DOF + DN],
                    in1=src[:, 0:4, DOF:DOF + DN], op=mx)
                nc.vector.tensor_tensor(
                    out=_rap(t1, DOF, [[5 * SLOT, 2], [1, DN]]),
                    in0=_rap(src, SLOT + DOF, [[-SLOT, 2], [1, DN]]),
                    in1=_rap(src, 5 * SLOT + DOF, [[-SLOT, 2], [1, DN]]), op=mx)
                if it == 0:
                    x_pair(src, 4, 5)
                else:
                    x_pair(src, 1, 2)
                    x_pair(src, 4, 5)
                # Z = max(X, M2) in place in t0
                nc.vector.tensor_tensor(out=t0[dv], in0=t0[dv], in1=t1[dv], op=mx)
                # Y' = Z + G_it (pure adds at fp16 2x rate); boundary rows
                # (1, 19) first so the halo DMAs overlap the interior build
                goff = it * 6 * SLOT
                wa = _rap(wb, 26, [[SLOT, 6], [468, 2], [1, 26]])
                za = _rap(t0, 26, [[SLOT, 6], [468, 2], [1, 26]])
                ga = _rap(gs, goff + 26, [[SLOT, 6], [468, 2], [1, 26]])
                nc.vector.tensor_tensor(out=wa, in0=za, in1=ga, op=add)
                if it < n_iter - 1:
                    halo()
                    # stream the G tile two iterations ahead, after the halo
                    # DMAs so it can never delay them in the engine FIFOs
                    if it + 2 < n_iter:
                        nc.sync.dma_start(out=gs[:, it + 2], in_=gs_d[:, it + 2])
                    wi = _rap(wb, 52, [[SLOT, 6], [1, 442]])
                    zi = _rap(t0, 52, [[SLOT, 6], [1, 442]])
                    gi = _rap(gs, goff + 52, [[SLOT, 6], [1, 442]])
                    nc.vector.tensor_tensor(out=wi, in0=zi, in1=gi, op=add)
                else:
                    # final iteration: build + ship out in slot halves so
                    # the two output DMAs overlap the remaining adds
                    for s0 in (0, 3):
                        wi = _rap(wb, s0 * SLOT + 52, [[SLOT, 3], [1, 442]])
                        zi = _rap(t0, s0 * SLOT + 52, [[SLOT, 3], [1, 442]])
                        gi = _rap(gs, goff + s0 * SLOT + 52, [[SLOT, 3], [1, 442]])
                        nc.vector.tensor_tensor(out=wi, in0=zi, in1=gi, op=add)
                        eng = nc.sync if s0 == 0 else nc.scalar
                        eng.dma_start(out=w_d[:, s0:s0 + 3], in_=wb[:, s0:s0 + 3])
    return nc


_NC_CACHE = {}


def _get_nc(n_iter):
    if n_iter not in _NC_CACHE:
        _NC_CACHE[n_iter] = build_nc(n_iter)
    return _NC_CACHE[n_iter]


# ---------------------------------------------------------------- host side
def _to_padded_axial(x):
    out = np.zeros(x.shape[:-2] + (PR, PC), np.float32)
    out[..., _ROW, _COL] = x
    return out


def kernel(offset_input_goals, offset_current_state, offset_obstacles,
           num_iterations):
    global LAST_RESULT
    goals = np.asarray(offset_input_goals, np.float32)
    state = np.asarray(offset_current_state)
    obst = np.asarray(offset_obstacles, np.float32)
    n_iter = int(num_iterations)
    B = goals.shape[0]
    assert B == N_CORES * BPC and n_iter >= 1

    goals_ax = _to_padded_axial(goals)                     # [B,6,40,26]
    mask = _to_padded_axial(np.ones((E, E), np.float32))
    m_full = (1.0 - _to_padded_axial(obst)) * mask         # [B,40,26]
    gmb_full = np.where(m_full[:, None] > 0.5, goals_ax, BIG)  # [B,6,40,26]

    def split(x):  # [B, ..., 40, 26] -> [B, ..., 546] halves
        h0 = x[..., 0:21, :].reshape(x.shape[:-2] + (SLOT,))
        h1 = x[..., 19:40, :].reshape(x.shape[:-2] + (SLOT,))
        return h0, h1

    g0, g1 = split(gmb_full)
    gmb_h = np.stack([g0, g1], 1)                          # [B,2,6,546]
    # rescaled-domain goal tensors: Y_k = W_k / gamma^(k+1) turns the
    # update into  Y' = max-tree(Y) + G_k,  G_k = gmb * gamma^-(k+1)
    scales = GAMMA ** -(np.arange(1, n_iter + 1, dtype=np.float32))
    gs_h = (gmb_h[:, :, None] * scales[None, None, :, None, None]).astype(NP_DT)

    in_maps = []
    for c in range(N_CORES):
        s = slice(c * BPC, (c + 1) * BPC)
        gi = np.concatenate([g0[s], g1[s]], 0).astype(NP_DT)
        gsc = np.concatenate([gs_h[s, 0], gs_h[s, 1]], 0)  # [128,n_iter,6,546]
        in_maps.append({"ginit": gi, "gs": gsc})

    nc = _get_nc(n_iter)
    res = run_bass_kernel_spmd(nc, in_maps, core_ids=list(range(N_CORES)),
                               trace=TRACE)
    LAST_RESULT = res

    w_all = np.stack([np.asarray(res.results[c]["w"], np.float32)
                      for c in range(N_CORES)], 0)         # [8,128,6,546]

    alpha = state[:, 0].astype(np.int64)
    uu = (state[:, 1] - state[:, 2] // 2 + ADD).astype(np.int64)  # grid row
    vv = state[:, 2].astype(np.int64)
    rot = (alpha + 1) % 6
    bs = np.arange(B)
    core = bs // BPC
    lane = bs % BPC

    w_scale = np.float32(GAMMA ** n_iter)   # W_final = Y_final * gamma^n

    def read_w(slot, g, c):
        # clamped gather of W at grid row g, col c (0 outside grid / masked)
        valid = (g >= 0) & (g <= 37) & (c >= 0) & (c <= 24)
        h = (g > 18).astype(np.int64)
        local = np.where(h == 1, g - 18, g + 1)
        p = h * 64 + lane
        idx = np.clip(local * 26 + c, 0, SLOT - 1)
        val = w_all[core, p, slot, idx] * w_scale
        return np.where(valid, np.maximum(val, 0.0), 0.0)

    dy0 = np.array([d[0] for d in D0])[rot]
    dx0 = np.array([d[1] for d in D0])[rot]
    m_pt = m_full[bs, uu + 1, vv]

    out = np.zeros((B, 4), np.float32)
    out[:, 0] = m_pt * read_w(rot, uu + dy0, vv + dx0)
    out[:, 1] = m_pt * read_w(rot, uu - dy0, vv - dx0)
    out[:, 2] = read_w((rot + 1) % 6, uu, vv)
    out[:, 3] = read_w((rot + 5) % 6, uu, vv)
    return out
